# revision 17
# baseline (speedup 1.0000x reference)
"""Cross-attention Trainium2 Bass kernel (nn_CrossAttention, B=4, Sq=Skv=2048,
query_dim=1024, kv_dim=768, H=16, D=64) on 8 NeuronCores.

The graded metric tracks wall-clock of kernel(); with axon-tunneled devices
that is dominated by host<->device transfer (~60-70 MB/s, serialized across
cores), so the design minimizes wire bytes: every input byte crosses the
tunnel exactly once, shared tensors are rebuilt on-device with collectives,
and lossy-compressible tensors are quantized within the 2e-2 error budget:
  - q and k ship as int8 with per-row f32 scales (their quantization error
    enters the softmax exponent and attenuates to ~0.2% on the output);
    dequantized to fp16 on-device during the upcast copy.
  - v and all weights ship as fp16 (their error passes linearly to the
    output, so int8 would blow the budget).
  - The output returns as int8 with per-row scales (~0.4% of row max).
Measured end-to-end relative error: ~5.4e-3 against the fp32 reference.

Sharding: core c -> head pair (2c, 2c+1), i.e. Q/K/V output dims
[128c, 128c+128) and Wo rows [128c, 128c+128).
  - Activations are sharded 1/8 per core on the wire and rebuilt on-device
    with world-group AllGathers (q|k packed [8192, 1792] i8, v [8192,768]).
  - Each core ships only its pair's weight columns (Wq/Wk/Wv cols, Wo rows)
    -- private, no collective needed.
  - Each core computes its 2 heads over ALL batches and writes a partial
    out (ctx_pair @ Wo_pair) to DRAM; a world ReduceScatter sums the 8
    partials and leaves rows [1024c, 1024(c+1)) on core c, which are
    bias-added, row-max int8-quantized, and returned.
  Only world-group (8-core) collectives are used: sub-group collectives
  crash the axon terminal when other jax executables run between calls.

Device pipeline (fp16 operands, f32 PSUM accumulation):
  - Activations arrive natural [seq, dim]; PE-transposes (identity matmul)
    build the feature-major copies the projections need.
  - Attention per batch as in the tuned baseline: scores transposed
    (S^T = K_h @ Q_h^T) so softmax's kv axis is on partitions, one 1024-wide
    exp per jc covers both heads, V augmented with a ones column so the
    softmax denominator falls out of the ctx matmul, ctx matmuls trail one
    jc behind the exp (software pipeline).
  - V bias is folded into bias_eff = bo + bv @ Wo (exact: softmax rows sum
    to 1), added after the ReduceScatter.
"""

import sys

sys.path.insert(0, "/opt/trn_rl_repo")

import numpy as np

import jax

# Persistent XLA compilation cache: run_bass_kernel_spmd re-jits its shard_map
# wrapper on every call; with the cache the recompile becomes a fast
# deserialization (saves ~0.25s per kernel() call).
jax.config.update("jax_compilation_cache_dir", "/tmp/jax_comp_cache")
jax.config.update("jax_persistent_cache_min_compile_time_secs", 0.0)
jax.config.update("jax_persistent_cache_min_entry_size_bytes", 0)

import concourse.bass as bass  # noqa: F401
import concourse.tile as tile
from concourse import bacc, mybir
from concourse.bass_utils import run_bass_kernel_spmd

F16 = mybir.dt.float16
F32 = mybir.dt.float32
I8 = mybir.dt.int8
EXP = mybir.ActivationFunctionType.Exp

B = 4
SQ = 2048
SKV = 2048
QDIM = 1024
KVDIM = 768
D = 64
KQ = QDIM // 128  # 8
KKV = KVDIM // 128  # 6
NB = 512  # q-block size for attention
VCOL = D + 1  # 65, V columns incl. ones
SQG = B * SQ  # 8192 global q rows
SQS = SQG // 8  # 1024 q rows shipped per core
KVG = B * SKV  # 8192 global kv rows (per tensor)
KVS = 2 * KVG // 8  # 2048 [K;V] rows shipped per core
WPR = QDIM + 2 * KVDIM + KQ * 128  # 3584 rows of the per-pair weight pack


def build_program():
    nc = bacc.Bacc("TRN2", target_bir_lowering=False, debug=False)

    qksh_d = nc.dram_tensor("qksh", [SQS, QDIM + KVDIM], I8, kind="ExternalInput")
    qksc_d = nc.dram_tensor("qksc", [SQS, 2], F32, kind="ExternalInput")
    vsh_d = nc.dram_tensor("vsh", [SQS, KVDIM], F16, kind="ExternalInput")
    wp_d = nc.dram_tensor("wp", [WPR, 128], F16, kind="ExternalInput")
    aux_d = nc.dram_tensor("aux", [2, 128], F32, kind="ExternalInput")
    beff_d = nc.dram_tensor("beff", [1, QDIM], F32, kind="ExternalInput")
    idn_d = nc.dram_tensor("idn", [128, 128], F16, kind="ExternalInput")
    out_d = nc.dram_tensor("out", [SQS, QDIM], I8, kind="ExternalOutput")
    osc_d = nc.dram_tensor("osc", [SQS, 1], F32, kind="ExternalOutput")

    n_jc = SKV // 128  # 16
    n_qb = SQ // NB  # 4 q-blocks per batch
    s_scale = 1.0 / np.sqrt(D)

    with tile.TileContext(nc) as tc:
        with (
            tc.tile_pool(name="sb", bufs=1) as sb,
            tc.tile_pool(name="ps", bufs=1, space="PSUM") as ps,
            tc.tile_pool(name="dram", bufs=1, space="DRAM") as dram,
        ):
            # ---- world-group gathers: fire first on gpsimd ----
            qkb = dram.tile([SQS, QDIM + KVDIM], I8, name="qkb")
            qkg = dram.tile([SQG, QDIM + KVDIM], I8, addr_space="Shared", name="qkg")
            scb = dram.tile([SQS, 2], F32, name="scb")
            scg = dram.tile([SQG, 2], F32, addr_space="Shared", name="scg")
            vb_t = dram.tile([SQS, KVDIM], F16, name="vb_t")
            vg = dram.tile([KVG, KVDIM], F16, addr_space="Shared", name="vg")
            pout = dram.tile([SQG, QDIM], F32, name="pout")
            rsout = dram.tile([SQS, QDIM], F32, name="rsout")
            nc.gpsimd.dma_start(scb[:], qksc_d.ap())
            nc.gpsimd.collective_compute(
                "AllGather",
                mybir.AluOpType.bypass,
                replica_groups=[list(range(8))],
                ins=[scb.opt()],
                outs=[scg.opt()],
            )
            nc.gpsimd.dma_start(qkb[:], qksh_d.ap())
            nc.gpsimd.collective_compute(
                "AllGather",
                mybir.AluOpType.bypass,
                replica_groups=[list(range(8))],
                ins=[qkb.opt()],
                outs=[qkg.opt()],
            )
            nc.gpsimd.dma_start(vb_t[:], vsh_d.ap())
            nc.gpsimd.collective_compute(
                "AllGather",
                mybir.AluOpType.bypass,
                replica_groups=[list(range(8))],
                ins=[vb_t.opt()],
                outs=[vg.opt()],
            )

            idn = sb.tile([128, 128], F16, tag="idn")
            nc.sync.dma_start(idn, idn_d.ap())
            ones_f32 = sb.tile([128, 1], F32, tag="ones")
            nc.vector.memset(ones_f32, 1.0)

            # ---- per-pair weights (local, no collective) ----
            wq_sb = sb.tile([128, KQ, 128], F16, tag="wq")
            for kc in range(KQ):
                nc.sync.dma_start(
                    wq_sb[:, kc, :], wp_d.ap()[kc * 128 : (kc + 1) * 128, :]
                )
            wk_sb = sb.tile([128, KKV, 128], F16, tag="wk")
            wv_sb = sb.tile([128, KKV, 128], F16, tag="wv")
            for kc in range(KKV):
                r0 = QDIM + kc * 128
                nc.sync.dma_start(wk_sb[:, kc, :], wp_d.ap()[r0 : r0 + 128, :])
                r0 = QDIM + KVDIM + kc * 128
                nc.sync.dma_start(wv_sb[:, kc, :], wp_d.ap()[r0 : r0 + 128, :])
            # Wo pair rows [128, 1024], packed row-major into wp rows 2560:3584
            wo_sb = sb.tile([128, QDIM], F16, tag="wo")
            nc.sync.dma_start(
                wo_sb,
                wp_d.ap()[QDIM + 2 * KVDIM :, :].rearrange(
                    "(p e) c -> p (e c)", e=KQ
                ),
            )
            bq_p = sb.tile([128, 1], F32, tag="bqp")
            nc.sync.dma_start(bq_p, aux_d.ap()[0:1].rearrange("o p -> p o"))
            bk_p = sb.tile([128, 1], F32, tag="bkp")
            nc.sync.dma_start(bk_p, aux_d.ap()[1:2].rearrange("o p -> p o"))
            be_sb = sb.tile([1, QDIM], F32, tag="be")
            nc.sync.dma_start(be_sb, beff_d.ap())
            be_bcast = sb.tile([128, QDIM], F32, tag="beb")
            nc.sync.dma_start(
                be_bcast, be_sb[0:1, None, :].to_broadcast((1, 128, QDIM))
            )

            def transpose_block(out_ps, in_sb):
                nc.tensor.matmul(
                    out_ps,
                    in_sb,
                    idn,
                    is_transpose=True,
                    start=True,
                    stop=True,
                    skip_group_check=True,
                )

            def load_k_rows(row0):
                """Load [128, 768] natural int8 k rows, dequant to fp16."""
                xn8 = sb.tile([128, KVDIM], I8, tag="ldn8", bufs=3, name="xn8")
                nc.sync.dma_start(xn8, qkg[row0 : row0 + 128, QDIM:])
                xsc = sb.tile([128, 1], F32, tag="xsc", bufs=3, name="xsc")
                nc.sync.dma_start(xsc, scg[row0 : row0 + 128, 1:2])
                xn = sb.tile([128, KVDIM], F16, tag="ldn", bufs=3, name="xn")
                nc.vector.tensor_scalar_mul(xn, xn8, xsc[:, 0:1])
                return xn

            def load_v_rows(row0):
                """Load [128, 768] natural fp16 v rows."""
                xn = sb.tile([128, KVDIM], F16, tag="ldn", bufs=3, name="xn")
                nc.sync.dma_start(xn, vg[row0 : row0 + 128, :])
                return xn

            # ---- per batch: transpose + project + attention + partial out ----
            for b in range(B):
                kt_b = sb.tile([128, SKV], F16, tag="ktb", bufs=2, name="kt_b")
                v_b = sb.tile(
                    [128, n_jc, 2 * VCOL], F16, tag="vb", bufs=2, name="v_b"
                )
                for jo in range(n_jc):
                    nc.vector.tensor_copy(
                        v_b[:, jo, :].rearrange("p (h d) -> p h d", d=VCOL)[
                            :, :, D : D + 1
                        ],
                        ones_f32[:, 0:1].to_broadcast((128, 2, 1)),
                    )
                for s2 in range(SKV // 1024):
                    # K: transpose rows kvg[2048b + 1024*s2 ...] then project
                    ktr = sb.tile(
                        [128, KKV, 1024], F16, tag="trc", bufs=2, name="ktr"
                    )
                    for r in range(8):
                        kn = load_k_rows(b * SKV + s2 * 1024 + r * 128)
                        trp = ps.tile([128, 512], F16, tag="mm", bufs=2, name="trpk")
                        for j4 in range(4):
                            transpose_block(
                                trp[:, j4 * 128 : (j4 + 1) * 128],
                                kn[:, j4 * 128 : (j4 + 1) * 128],
                            )
                        nc.vector.tensor_copy(
                            ktr[:, 0:4, r * 128 : (r + 1) * 128],
                            trp.rearrange("p (j s) -> p j s", s=128),
                        )
                        trp2 = ps.tile(
                            [128, 512], F16, tag="mm", bufs=2, name="trpk2"
                        )
                        for j4 in range(2):
                            transpose_block(
                                trp2[:, j4 * 128 : (j4 + 1) * 128],
                                kn[:, (4 + j4) * 128 : (5 + j4) * 128],
                            )
                        nc.vector.tensor_copy(
                            ktr[:, 4:6, r * 128 : (r + 1) * 128],
                            trp2[:, 0:256].rearrange("p (j s) -> p j s", s=128),
                        )
                    kps = ps.tile([128, 1024], F32, tag="st", bufs=2, name="kps")
                    for kc in range(KKV):
                        for hs in range(2):
                            nc.tensor.matmul(
                                kps[:, hs * 512 : (hs + 1) * 512],
                                wk_sb[:, kc, :],
                                ktr[:, kc, hs * 512 : (hs + 1) * 512],
                                start=(kc == 0),
                                stop=(kc == KKV - 1),
                                skip_group_check=True,
                            )
                    nc.vector.tensor_scalar_add(
                        out=kt_b[:, s2 * 1024 : (s2 + 1) * 1024],
                        in0=kps,
                        scalar1=bk_p[:, 0:1],
                    )
                    # V: transpose rows kvg[8192 + 2048b + ...] then project
                    vtr = sb.tile(
                        [128, KKV, 1024], F16, tag="trc", bufs=2, name="vtr"
                    )
                    for r in range(8):
                        vn = load_v_rows(b * SKV + s2 * 1024 + r * 128)
                        trp = ps.tile([128, 512], F16, tag="mm", bufs=2, name="trpv")
                        for j4 in range(4):
                            transpose_block(
                                trp[:, j4 * 128 : (j4 + 1) * 128],
                                vn[:, j4 * 128 : (j4 + 1) * 128],
                            )
                        nc.vector.tensor_copy(
                            vtr[:, 0:4, r * 128 : (r + 1) * 128],
                            trp.rearrange("p (j s) -> p j s", s=128),
                        )
                        trp2 = ps.tile(
                            [128, 512], F16, tag="mm", bufs=2, name="trpv2"
                        )
                        for j4 in range(2):
                            transpose_block(
                                trp2[:, j4 * 128 : (j4 + 1) * 128],
                                vn[:, (4 + j4) * 128 : (5 + j4) * 128],
                            )
                        nc.vector.tensor_copy(
                            vtr[:, 4:6, r * 128 : (r + 1) * 128],
                            trp2[:, 0:256].rearrange("p (j s) -> p j s", s=128),
                        )
                    # V proj (no bias; folded into bias_eff): natural layout
                    for r in range(8):
                        jo = s2 * 8 + r
                        vps = ps.tile([128, 128], F32, tag="mm", bufs=2, name="vps")
                        for kc in range(KKV):
                            nc.tensor.matmul(
                                vps,
                                vtr[:, kc, r * 128 : (r + 1) * 128],
                                wv_sb[:, kc, :],
                                start=(kc == 0),
                                stop=(kc == KKV - 1),
                                skip_group_check=True,
                            )
                        nc.vector.tensor_copy(
                            v_b[:, jo, :].rearrange("p (h d) -> p h d", d=VCOL)[
                                :, :, 0:D
                            ],
                            vps.rearrange("p (h d) -> p h d", d=D),
                        )

                # Q: transpose rows qg[2048b:...] then project (+bq)
                qt_b = sb.tile([128, SQ], F16, tag="qtb", bufs=2, name="qt_b")
                for s2 in range(SQ // 1024):
                    qtr = sb.tile(
                        [128, KQ, 1024], F16, tag="qtr", bufs=2, name="qtr"
                    )
                    for r in range(8):
                        row0 = b * SQ + s2 * 1024 + r * 128
                        qn8 = sb.tile([128, QDIM], I8, tag="ldn8", bufs=3, name="qn8")
                        nc.sync.dma_start(qn8, qkg[row0 : row0 + 128, 0:QDIM])
                        qsc = sb.tile([128, 1], F32, tag="xsc", bufs=3, name="qsc")
                        nc.sync.dma_start(qsc, scg[row0 : row0 + 128, 0:1])
                        qn = sb.tile([128, QDIM], F16, tag="ldn", bufs=3, name="qn")
                        nc.vector.tensor_scalar_mul(qn, qn8, qsc[:, 0:1])
                        for hf in range(2):
                            trp = ps.tile(
                                [128, 512], F16, tag="mm", bufs=2, name="trpq"
                            )
                            for j4 in range(4):
                                j = hf * 4 + j4
                                transpose_block(
                                    trp[:, j4 * 128 : (j4 + 1) * 128],
                                    qn[:, j * 128 : (j + 1) * 128],
                                )
                            nc.vector.tensor_copy(
                                qtr[
                                    :,
                                    hf * 4 : (hf + 1) * 4,
                                    r * 128 : (r + 1) * 128,
                                ],
                                trp.rearrange("p (j s) -> p j s", s=128),
                            )
                    qps = ps.tile([128, 1024], F32, tag="st", bufs=2, name="qps")
                    for kc in range(KQ):
                        for hs in range(2):
                            nc.tensor.matmul(
                                qps[:, hs * 512 : (hs + 1) * 512],
                                wq_sb[:, kc, :],
                                qtr[:, kc, hs * 512 : (hs + 1) * 512],
                                start=(kc == 0),
                                stop=(kc == KQ - 1),
                                skip_group_check=True,
                            )
                    nc.vector.tensor_scalar_add(
                        out=qt_b[:, s2 * 1024 : (s2 + 1) * 1024],
                        in0=qps,
                        scalar1=bq_p[:, 0:1],
                    )

                # attention + partial out-proj per q-block
                for qb in range(n_qb):
                    qsl = slice(qb * NB, (qb + 1) * NB)
                    ctx_a = ps.tile([128, NB], F32, tag="ctx", bufs=2, name="ctx_a")
                    ctx_b = ps.tile([128, NB], F32, tag="ctx", bufs=2, name="ctx_b")
                    e_prev = None
                    for jc in range(n_jc):
                        st_ps = ps.tile(
                            [128, 2 * NB], F32, tag="st", bufs=2, name="st_ps"
                        )
                        jsl = slice(jc * 128, (jc + 1) * 128)
                        nc.tensor.matmul(
                            st_ps[:, 0:NB],
                            kt_b[0:64, jsl],
                            qt_b[0:64, qsl],
                            start=True,
                            stop=True,
                            skip_group_check=True,
                        )
                        nc.tensor.matmul(
                            st_ps[:, NB : 2 * NB],
                            kt_b[64:128, jsl],
                            qt_b[64:128, qsl],
                            start=True,
                            stop=True,
                            skip_group_check=True,
                        )
                        e_t = sb.tile(
                            [128, 2 * NB], F16, tag="e", bufs=2, name="e_t"
                        )
                        nc.scalar.activation(
                            out=e_t, in_=st_ps, func=EXP, scale=s_scale
                        )
                        if e_prev is not None:
                            pj = jc - 1
                            nc.tensor.matmul(
                                ctx_a[0:VCOL, :],
                                v_b[:, pj, 0:VCOL],
                                e_prev[:, 0:NB],
                                start=(pj == 0),
                                stop=False,
                                skip_group_check=True,
                            )
                            nc.tensor.matmul(
                                ctx_b[0:VCOL, :],
                                v_b[:, pj, VCOL : 2 * VCOL],
                                e_prev[:, NB : 2 * NB],
                                start=(pj == 0),
                                stop=False,
                                skip_group_check=True,
                            )
                        e_prev = e_t
                    pj = n_jc - 1
                    nc.tensor.matmul(
                        ctx_a[0:VCOL, :],
                        v_b[:, pj, 0:VCOL],
                        e_prev[:, 0:NB],
                        start=False,
                        stop=True,
                        skip_group_check=True,
                    )
                    nc.tensor.matmul(
                        ctx_b[0:VCOL, :],
                        v_b[:, pj, VCOL : 2 * VCOL],
                        e_prev[:, NB : 2 * NB],
                        start=False,
                        stop=True,
                        skip_group_check=True,
                    )
                    # normalization: denominators at psum row 64
                    stage = sb.tile(
                        [128, NB], F32, tag="stage", bufs=1, name="stage"
                    )
                    nc.vector.tensor_copy(stage[64:65, :], ctx_a[64:65, :])
                    nc.vector.tensor_copy(stage[96:97, :], ctx_b[64:65, :])
                    ctxu = sb.tile([128, NB], F32, tag="ctxu", bufs=2, name="ctxu")
                    nc.vector.tensor_copy(ctxu[0:64, :], ctx_a[0:64, :])
                    nc.vector.tensor_copy(ctxu[64:128, :], ctx_b[0:64, :])
                    sums_p = sb.tile([2, NB], F32, tag="sums", bufs=1, name="sums_p")
                    nc.sync.dma_start(sums_p[0:1, :], stage[64:65, :])
                    nc.sync.dma_start(sums_p[1:2, :], stage[96:97, :])
                    rsum_p = sb.tile([2, NB], F32, tag="rsum", bufs=1, name="rsum_p")
                    nc.vector.reciprocal(out=rsum_p, in_=sums_p)
                    rb = sb.tile([128, NB], F32, tag="rb", bufs=1, name="rb")
                    for sub in range(2):
                        nc.sync.dma_start(
                            rb[sub * 64 : sub * 64 + 64, :],
                            rsum_p[sub : sub + 1, None, :].to_broadcast((1, 64, NB)),
                        )
                    ctxn = sb.tile([128, NB], F16, tag="ctxn", bufs=2, name="ctxn")
                    nc.vector.tensor_mul(out=ctxn, in0=ctxu, in1=rb)

                    # partial out rows -> pout (no bias here; added post-RS)
                    for sti in range(NB // 128):
                        ops = ps.tile([128, QDIM], F32, tag="st", bufs=2, name="ops")
                        for hs in range(2):
                            nc.tensor.matmul(
                                ops[:, hs * 512 : (hs + 1) * 512],
                                ctxn[:, sti * 128 : (sti + 1) * 128],
                                wo_sb[:, hs * 512 : (hs + 1) * 512],
                                start=True,
                                stop=True,
                                skip_group_check=True,
                            )
                        posb = sb.tile(
                            [128, QDIM], F32, tag="posb", bufs=2, name="posb"
                        )
                        nc.vector.tensor_copy(posb, ops)
                        r0 = b * SQ + qb * NB + sti * 128
                        nc.sync.dma_start(pout[r0 : r0 + 128, :], posb)

            # ---- world ReduceScatter of partial outs -> my 1024 rows ----
            nc.gpsimd.collective_compute(
                "ReduceScatter",
                mybir.AluOpType.add,
                replica_groups=[list(range(8))],
                ins=[pout.opt()],
                outs=[rsout.opt()],
            )

            # ---- bias add + int8 row-scale quantization ----
            for i in range(SQS // 128):
                rsf = sb.tile([128, QDIM], F32, tag="posb", bufs=2, name="rsf")
                nc.sync.dma_start(rsf, rsout[i * 128 : (i + 1) * 128, :])
                osf = sb.tile([128, QDIM], F32, tag="osf", bufs=2, name="osf")
                nc.vector.tensor_add(osf, rsf, be_bcast)
                am = sb.tile([128, 1], F32, tag="am", bufs=2, name="am")
                nc.vector.tensor_reduce(
                    out=am,
                    in_=osf,
                    axis=mybir.AxisListType.X,
                    op=mybir.AluOpType.max,
                    apply_absolute_value=True,
                )
                nc.vector.tensor_scalar_max(am, am, 1e-30)
                rr = sb.tile([128, 1], F32, tag="rr", bufs=2, name="rr")
                nc.vector.reciprocal(out=rr, in_=am)
                osb = sb.tile([128, QDIM], I8, tag="osb", bufs=2, name="osb")
                nc.vector.tensor_scalar(
                    out=osb,
                    in0=osf,
                    scalar1=rr[:, 0:1],
                    scalar2=126.0,
                    op0=mybir.AluOpType.mult,
                    op1=mybir.AluOpType.mult,
                )
                nc.sync.dma_start(out_d.ap()[i * 128 : (i + 1) * 128, :], osb)
                nc.sync.dma_start(osc_d.ap()[i * 128 : (i + 1) * 128, :], am)

    nc.compile()
    return nc


_NC_CACHE = {}


def _get_nc():
    if "nc" not in _NC_CACHE:
        _NC_CACHE["nc"] = build_program()
    return _NC_CACHE["nc"]


_IDN = np.eye(128, dtype=np.float16)


def _rowquant_i8(x):
    am = np.maximum(np.abs(x).max(1), 1e-30).astype(np.float32)
    xi = np.rint(x * (126.0 / am)[:, None]).astype(np.int8)
    return xi, (am * (1.0 / 126.0))


def make_in_maps(query, key, value, Wq, bq, Wk, bk, Wv, bv, Wo, bo):
    f16 = np.float16
    q_i8, qsc = _rowquant_i8(query.reshape(SQG, QDIM))
    k_i8, ksc = _rowquant_i8(key.reshape(KVG, KVDIM))
    qk_i8 = np.concatenate([q_i8, k_i8], axis=1)
    qksc = np.stack([qsc, ksc], axis=1).astype(np.float32)
    v16 = value.astype(f16).reshape(KVG, KVDIM)
    bias_eff = (
        bo.astype(np.float64) + bv.astype(np.float64) @ Wo.astype(np.float64)
    ).astype(np.float32)
    beff = bias_eff.reshape(1, QDIM)
    in_maps = []
    for c in range(8):
        cs = slice(c * 128, (c + 1) * 128)
        wpk = np.empty((WPR, 128), f16)
        wpk[0:QDIM] = Wq[:, cs]
        wpk[QDIM : QDIM + KVDIM] = Wk[:, cs]
        wpk[QDIM + KVDIM : QDIM + 2 * KVDIM] = Wv[:, cs]
        wpk[QDIM + 2 * KVDIM :] = (
            Wo[cs, :].astype(f16).reshape(KQ * 128, 128)
        )
        aux = np.stack([bq[cs], bk[cs]]).astype(np.float32)
        in_maps.append(
            dict(
                qksh=qk_i8[c * SQS : (c + 1) * SQS],
                qksc=qksc[c * SQS : (c + 1) * SQS],
                vsh=v16[c * SQS : (c + 1) * SQS],
                wp=wpk,
                aux=aux,
                beff=beff,
                idn=_IDN,
            )
        )
    return in_maps


def kernel(query, key, value, Wq, bq, Wk, bk, Wv, bv, Wo, bo, _trace=False):
    nc = _get_nc()
    in_maps = make_in_maps(query, key, value, Wq, bq, Wk, bk, Wv, bv, Wo, bo)
    try:
        res = run_bass_kernel_spmd(
            nc, in_maps, core_ids=list(range(8)), trace=_trace
        )
    except Exception:
        # one retry for transient axon-tunnel drops
        res = run_bass_kernel_spmd(
            nc, in_maps, core_ids=list(range(8)), trace=_trace
        )
    out = np.empty((SQG, QDIM), np.float32)
    for c in range(8):
        sc = res.results[c]["osc"].astype(np.float32) * (1.0 / 126.0)
        np.multiply(
            res.results[c]["out"],
            sc,
            out=out[c * SQS : (c + 1) * SQS],
            dtype=np.float32,
        )
    out = out.reshape(B, SQ, QDIM)
    if _trace:
        return out, res
    return out


# revision 18
# speedup vs baseline: 1.0303x; 1.0303x over previous
"""Cross-attention Trainium2 Bass kernel (nn_CrossAttention, B=4, Sq=Skv=2048,
query_dim=1024, kv_dim=768, H=16, D=64) on 8 NeuronCores.

The graded metric tracks wall-clock of kernel(); with axon-tunneled devices
that is dominated by host<->device transfer (~60-70 MB/s, serialized across
cores), so the design minimizes wire bytes: every input byte crosses the
tunnel exactly once, shared tensors are rebuilt on-device with collectives,
and lossy-compressible tensors are quantized within the 2e-2 error budget:
  - q and k ship as int8 with per-row f32 scales (their quantization error
    enters the softmax exponent and attenuates to ~0.2% on the output);
    dequantized to fp16 on-device during the upcast copy.
  - v and all weights ship as fp16 (their error passes linearly to the
    output, so int8 would blow the budget).
  - The output returns as int8 with per-row scales (~0.4% of row max).
Measured end-to-end relative error: ~5.4e-3 against the fp32 reference.

Sharding: core c -> head pair (2c, 2c+1), i.e. Q/K/V output dims
[128c, 128c+128) and Wo rows [128c, 128c+128).
  - Activations are sharded 1/8 per core on the wire and rebuilt on-device
    with world-group AllGathers (q|k packed [8192, 1792] i8, v [8192,768]).
  - Each core ships only its pair's weight columns (Wq/Wk/Wv cols, Wo rows)
    -- private, no collective needed.
  - Each core computes its 2 heads over ALL batches and writes a partial
    out (ctx_pair @ Wo_pair) to DRAM; a world ReduceScatter sums the 8
    partials and leaves rows [1024c, 1024(c+1)) on core c, which are
    bias-added, row-max int8-quantized, and returned.
  Only world-group (8-core) collectives are used: sub-group collectives
  crash the axon terminal when other jax executables run between calls.

Device pipeline (fp16 operands, f32 PSUM accumulation):
  - Activations arrive natural [seq, dim]; PE-transposes (identity matmul)
    build the feature-major copies the projections need.
  - Attention per batch as in the tuned baseline: scores transposed
    (S^T = K_h @ Q_h^T) so softmax's kv axis is on partitions, one 1024-wide
    exp per jc covers both heads, V augmented with a ones column so the
    softmax denominator falls out of the ctx matmul, ctx matmuls trail one
    jc behind the exp (software pipeline).
  - V bias is folded into bias_eff = bo + bv @ Wo (exact: softmax rows sum
    to 1), added after the ReduceScatter.
"""

import sys

sys.path.insert(0, "/opt/trn_rl_repo")

import numpy as np

import jax

# Persistent XLA compilation cache: run_bass_kernel_spmd re-jits its shard_map
# wrapper on every call; with the cache the recompile becomes a fast
# deserialization (saves ~0.25s per kernel() call).
jax.config.update("jax_compilation_cache_dir", "/tmp/jax_comp_cache")
jax.config.update("jax_persistent_cache_min_compile_time_secs", 0.0)
jax.config.update("jax_persistent_cache_min_entry_size_bytes", 0)

import concourse.bass as bass  # noqa: F401
import concourse.tile as tile
from concourse import bacc, mybir
from concourse.bass_utils import run_bass_kernel_spmd

F16 = mybir.dt.float16
F32 = mybir.dt.float32
I8 = mybir.dt.int8
EXP = mybir.ActivationFunctionType.Exp

B = 4
SQ = 2048
SKV = 2048
QDIM = 1024
KVDIM = 768
D = 64
KQ = QDIM // 128  # 8
KKV = KVDIM // 128  # 6
NB = 512  # q-block size for attention
VCOL = D + 1  # 65, V columns incl. ones
SQG = B * SQ  # 8192 global q rows
SQS = SQG // 8  # 1024 q rows shipped per core
KVG = B * SKV  # 8192 global kv rows (per tensor)
KVS = 2 * KVG // 8  # 2048 [K;V] rows shipped per core
WPR = QDIM + 2 * KVDIM + KQ * 128  # 3584 rows of the per-pair weight pack


def build_program():
    nc = bacc.Bacc("TRN2", target_bir_lowering=False, debug=False)

    qksh_d = nc.dram_tensor("qksh", [SQS, QDIM + KVDIM], I8, kind="ExternalInput")
    qksc_d = nc.dram_tensor("qksc", [SQS, 2], F32, kind="ExternalInput")
    vsh_d = nc.dram_tensor("vsh", [SQS, KVDIM], F16, kind="ExternalInput")
    wp_d = nc.dram_tensor("wp", [WPR, 128], F16, kind="ExternalInput")
    aux_d = nc.dram_tensor("aux", [2, 128], F32, kind="ExternalInput")
    beff_d = nc.dram_tensor("beff", [1, QDIM], F32, kind="ExternalInput")
    idn_d = nc.dram_tensor("idn", [128, 128], F16, kind="ExternalInput")
    out_d = nc.dram_tensor("out", [SQS, QDIM], I8, kind="ExternalOutput")
    osc_d = nc.dram_tensor("osc", [SQS, 1], F32, kind="ExternalOutput")

    n_jc = SKV // 128  # 16
    n_qb = SQ // NB  # 4 q-blocks per batch
    s_scale = 1.0 / np.sqrt(D)

    with tile.TileContext(nc) as tc:
        with (
            tc.tile_pool(name="sb", bufs=1) as sb,
            tc.tile_pool(name="ps", bufs=1, space="PSUM") as ps,
            tc.tile_pool(name="dram", bufs=1, space="DRAM") as dram,
        ):
            # ---- world-group gathers: fire first on gpsimd ----
            qkb = dram.tile([SQS, QDIM + KVDIM], I8, name="qkb")
            qkg = dram.tile([SQG, QDIM + KVDIM], I8, addr_space="Shared", name="qkg")
            scb = dram.tile([SQS, 2], F32, name="scb")
            scg = dram.tile([SQG, 2], F32, addr_space="Shared", name="scg")
            vb_t = dram.tile([SQS, KVDIM], F16, name="vb_t")
            vg = dram.tile([KVG, KVDIM], F16, addr_space="Shared", name="vg")
            pout = dram.tile([SQG, QDIM], F32, name="pout")
            rsout = dram.tile([SQS, QDIM], F32, name="rsout")
            nc.gpsimd.dma_start(scb[:], qksc_d.ap())
            nc.gpsimd.collective_compute(
                "AllGather",
                mybir.AluOpType.bypass,
                replica_groups=[list(range(8))],
                ins=[scb.opt()],
                outs=[scg.opt()],
            )
            nc.gpsimd.dma_start(qkb[:], qksh_d.ap())
            nc.gpsimd.collective_compute(
                "AllGather",
                mybir.AluOpType.bypass,
                replica_groups=[list(range(8))],
                ins=[qkb.opt()],
                outs=[qkg.opt()],
            )
            nc.gpsimd.dma_start(vb_t[:], vsh_d.ap())
            nc.gpsimd.collective_compute(
                "AllGather",
                mybir.AluOpType.bypass,
                replica_groups=[list(range(8))],
                ins=[vb_t.opt()],
                outs=[vg.opt()],
            )

            idn = sb.tile([128, 128], F16, tag="idn")
            nc.sync.dma_start(idn, idn_d.ap())
            ones_f32 = sb.tile([128, 1], F32, tag="ones")
            nc.vector.memset(ones_f32, 1.0)

            # ---- per-pair weights (local, no collective) ----
            wq_sb = sb.tile([128, KQ, 128], F16, tag="wq")
            for kc in range(KQ):
                nc.sync.dma_start(
                    wq_sb[:, kc, :], wp_d.ap()[kc * 128 : (kc + 1) * 128, :]
                )
            wk_sb = sb.tile([128, KKV, 128], F16, tag="wk")
            wv_sb = sb.tile([128, KKV, 128], F16, tag="wv")
            for kc in range(KKV):
                r0 = QDIM + kc * 128
                nc.sync.dma_start(wk_sb[:, kc, :], wp_d.ap()[r0 : r0 + 128, :])
                r0 = QDIM + KVDIM + kc * 128
                nc.sync.dma_start(wv_sb[:, kc, :], wp_d.ap()[r0 : r0 + 128, :])
            # Wo pair rows [128, 1024], packed row-major into wp rows 2560:3584
            wo_sb = sb.tile([128, QDIM], F16, tag="wo")
            nc.sync.dma_start(
                wo_sb,
                wp_d.ap()[QDIM + 2 * KVDIM :, :].rearrange(
                    "(p e) c -> p (e c)", e=KQ
                ),
            )
            bq_p = sb.tile([128, 1], F32, tag="bqp")
            nc.sync.dma_start(bq_p, aux_d.ap()[0:1].rearrange("o p -> p o"))
            bk_p = sb.tile([128, 1], F32, tag="bkp")
            nc.sync.dma_start(bk_p, aux_d.ap()[1:2].rearrange("o p -> p o"))
            be_sb = sb.tile([1, QDIM], F32, tag="be")
            nc.sync.dma_start(be_sb, beff_d.ap())
            be_bcast = sb.tile([128, QDIM], F32, tag="beb")
            nc.sync.dma_start(
                be_bcast, be_sb[0:1, None, :].to_broadcast((1, 128, QDIM))
            )

            def transpose_block(out_ps, in_sb):
                nc.tensor.matmul(
                    out_ps,
                    in_sb,
                    idn,
                    is_transpose=True,
                    start=True,
                    stop=True,
                    skip_group_check=True,
                )

            def load_k_rows(row0):
                """Load [128, 768] natural int8 k rows, dequant to fp16."""
                xn8 = sb.tile([128, KVDIM], I8, tag="ldn8", bufs=3, name="xn8")
                nc.sync.dma_start(xn8, qkg[row0 : row0 + 128, QDIM:])
                xsc = sb.tile([128, 1], F32, tag="xsc", bufs=3, name="xsc")
                nc.sync.dma_start(xsc, scg[row0 : row0 + 128, 1:2])
                xn = sb.tile([128, KVDIM], F16, tag="ldn", bufs=3, name="xn")
                nc.vector.tensor_scalar_mul(xn, xn8, xsc[:, 0:1])
                return xn

            def load_v_rows(row0):
                """Load [128, 768] natural fp16 v rows."""
                xn = sb.tile([128, KVDIM], F16, tag="ldn", bufs=3, name="xn")
                nc.sync.dma_start(xn, vg[row0 : row0 + 128, :])
                return xn

            # ---- per batch: transpose + project + attention + partial out ----
            for b in range(B):
                kt_b = sb.tile([128, SKV], F16, tag="ktb", bufs=2, name="kt_b")
                v_b = sb.tile(
                    [128, n_jc, 2 * VCOL], F16, tag="vb", bufs=2, name="v_b"
                )
                for jo in range(n_jc):
                    nc.vector.tensor_copy(
                        v_b[:, jo, :].rearrange("p (h d) -> p h d", d=VCOL)[
                            :, :, D : D + 1
                        ],
                        ones_f32[:, 0:1].to_broadcast((128, 2, 1)),
                    )
                for s2 in range(SKV // 1024):
                    # K: transpose rows kvg[2048b + 1024*s2 ...] then project
                    ktr = sb.tile(
                        [128, KKV, 1024], F16, tag="trc", bufs=2, name="ktr"
                    )
                    for r in range(8):
                        kn = load_k_rows(b * SKV + s2 * 1024 + r * 128)
                        trp = ps.tile([128, 512], F16, tag="mm", bufs=2, name="trpk")
                        for j4 in range(4):
                            transpose_block(
                                trp[:, j4 * 128 : (j4 + 1) * 128],
                                kn[:, j4 * 128 : (j4 + 1) * 128],
                            )
                        nc.vector.tensor_copy(
                            ktr[:, 0:4, r * 128 : (r + 1) * 128],
                            trp.rearrange("p (j s) -> p j s", s=128),
                        )
                        trp2 = ps.tile(
                            [128, 512], F16, tag="mm", bufs=2, name="trpk2"
                        )
                        for j4 in range(2):
                            transpose_block(
                                trp2[:, j4 * 128 : (j4 + 1) * 128],
                                kn[:, (4 + j4) * 128 : (5 + j4) * 128],
                            )
                        nc.vector.tensor_copy(
                            ktr[:, 4:6, r * 128 : (r + 1) * 128],
                            trp2[:, 0:256].rearrange("p (j s) -> p j s", s=128),
                        )
                    kps = ps.tile([128, 1024], F32, tag="st", bufs=2, name="kps")
                    for kc in range(KKV):
                        for hs in range(2):
                            nc.tensor.matmul(
                                kps[:, hs * 512 : (hs + 1) * 512],
                                wk_sb[:, kc, :],
                                ktr[:, kc, hs * 512 : (hs + 1) * 512],
                                start=(kc == 0),
                                stop=(kc == KKV - 1),
                                skip_group_check=True,
                            )
                    nc.vector.tensor_scalar_add(
                        out=kt_b[:, s2 * 1024 : (s2 + 1) * 1024],
                        in0=kps,
                        scalar1=bk_p[:, 0:1],
                    )
                    # V: transpose rows kvg[8192 + 2048b + ...] then project
                    vtr = sb.tile(
                        [128, KKV, 1024], F16, tag="trc", bufs=2, name="vtr"
                    )
                    for r in range(8):
                        vn = load_v_rows(b * SKV + s2 * 1024 + r * 128)
                        trp = ps.tile([128, 512], F16, tag="mm", bufs=2, name="trpv")
                        for j4 in range(4):
                            transpose_block(
                                trp[:, j4 * 128 : (j4 + 1) * 128],
                                vn[:, j4 * 128 : (j4 + 1) * 128],
                            )
                        nc.vector.tensor_copy(
                            vtr[:, 0:4, r * 128 : (r + 1) * 128],
                            trp.rearrange("p (j s) -> p j s", s=128),
                        )
                        trp2 = ps.tile(
                            [128, 512], F16, tag="mm", bufs=2, name="trpv2"
                        )
                        for j4 in range(2):
                            transpose_block(
                                trp2[:, j4 * 128 : (j4 + 1) * 128],
                                vn[:, (4 + j4) * 128 : (5 + j4) * 128],
                            )
                        nc.vector.tensor_copy(
                            vtr[:, 4:6, r * 128 : (r + 1) * 128],
                            trp2[:, 0:256].rearrange("p (j s) -> p j s", s=128),
                        )
                    # V proj (no bias; folded into bias_eff): natural layout
                    for r in range(8):
                        jo = s2 * 8 + r
                        vps = ps.tile([128, 128], F32, tag="mm", bufs=2, name="vps")
                        for kc in range(KKV):
                            nc.tensor.matmul(
                                vps,
                                vtr[:, kc, r * 128 : (r + 1) * 128],
                                wv_sb[:, kc, :],
                                start=(kc == 0),
                                stop=(kc == KKV - 1),
                                skip_group_check=True,
                            )
                        nc.vector.tensor_copy(
                            v_b[:, jo, :].rearrange("p (h d) -> p h d", d=VCOL)[
                                :, :, 0:D
                            ],
                            vps.rearrange("p (h d) -> p h d", d=D),
                        )

                # Q: dequant + transpose rows qkg[2048b:..., 0:QDIM], then project (+bq)
                qt_b = sb.tile([128, SQ], F16, tag="qtb", bufs=2, name="qt_b")
                for s2 in range(SQ // 1024):
                    qtr = sb.tile(
                        [128, KQ, 1024], F16, tag="qtr", bufs=2, name="qtr"
                    )
                    for r in range(8):
                        row0 = b * SQ + s2 * 1024 + r * 128
                        qn8 = sb.tile([128, QDIM], I8, tag="ldn8", bufs=3, name="qn8")
                        nc.sync.dma_start(qn8, qkg[row0 : row0 + 128, 0:QDIM])
                        qsc = sb.tile([128, 1], F32, tag="xsc", bufs=3, name="qsc")
                        nc.sync.dma_start(qsc, scg[row0 : row0 + 128, 0:1])
                        qn = sb.tile([128, QDIM], F16, tag="ldn", bufs=3, name="qn")
                        nc.vector.tensor_scalar_mul(qn, qn8, qsc[:, 0:1])
                        for hf in range(2):
                            trp = ps.tile(
                                [128, 512], F16, tag="mm", bufs=2, name="trpq"
                            )
                            for j4 in range(4):
                                j = hf * 4 + j4
                                transpose_block(
                                    trp[:, j4 * 128 : (j4 + 1) * 128],
                                    qn[:, j * 128 : (j + 1) * 128],
                                )
                            nc.vector.tensor_copy(
                                qtr[
                                    :,
                                    hf * 4 : (hf + 1) * 4,
                                    r * 128 : (r + 1) * 128,
                                ],
                                trp.rearrange("p (j s) -> p j s", s=128),
                            )
                    qps = ps.tile([128, 1024], F32, tag="st", bufs=2, name="qps")
                    for kc in range(KQ):
                        for hs in range(2):
                            nc.tensor.matmul(
                                qps[:, hs * 512 : (hs + 1) * 512],
                                wq_sb[:, kc, :],
                                qtr[:, kc, hs * 512 : (hs + 1) * 512],
                                start=(kc == 0),
                                stop=(kc == KQ - 1),
                                skip_group_check=True,
                            )
                    nc.vector.tensor_scalar_add(
                        out=qt_b[:, s2 * 1024 : (s2 + 1) * 1024],
                        in0=qps,
                        scalar1=bq_p[:, 0:1],
                    )

                # attention + partial out-proj per q-block
                for qb in range(n_qb):
                    qsl = slice(qb * NB, (qb + 1) * NB)
                    ctx_a = ps.tile([128, NB], F32, tag="ctx", bufs=2, name="ctx_a")
                    ctx_b = ps.tile([128, NB], F32, tag="ctx", bufs=2, name="ctx_b")
                    e_prev = None
                    for jc in range(n_jc):
                        st_ps = ps.tile(
                            [128, 2 * NB], F32, tag="st", bufs=2, name="st_ps"
                        )
                        jsl = slice(jc * 128, (jc + 1) * 128)
                        nc.tensor.matmul(
                            st_ps[:, 0:NB],
                            kt_b[0:64, jsl],
                            qt_b[0:64, qsl],
                            start=True,
                            stop=True,
                            skip_group_check=True,
                        )
                        nc.tensor.matmul(
                            st_ps[:, NB : 2 * NB],
                            kt_b[64:128, jsl],
                            qt_b[64:128, qsl],
                            start=True,
                            stop=True,
                            skip_group_check=True,
                        )
                        e_t = sb.tile(
                            [128, 2 * NB], F16, tag="e", bufs=2, name="e_t"
                        )
                        nc.scalar.activation(
                            out=e_t, in_=st_ps, func=EXP, scale=s_scale
                        )
                        if e_prev is not None:
                            pj = jc - 1
                            nc.tensor.matmul(
                                ctx_a[0:VCOL, :],
                                v_b[:, pj, 0:VCOL],
                                e_prev[:, 0:NB],
                                start=(pj == 0),
                                stop=False,
                                skip_group_check=True,
                            )
                            nc.tensor.matmul(
                                ctx_b[0:VCOL, :],
                                v_b[:, pj, VCOL : 2 * VCOL],
                                e_prev[:, NB : 2 * NB],
                                start=(pj == 0),
                                stop=False,
                                skip_group_check=True,
                            )
                        e_prev = e_t
                    pj = n_jc - 1
                    nc.tensor.matmul(
                        ctx_a[0:VCOL, :],
                        v_b[:, pj, 0:VCOL],
                        e_prev[:, 0:NB],
                        start=False,
                        stop=True,
                        skip_group_check=True,
                    )
                    nc.tensor.matmul(
                        ctx_b[0:VCOL, :],
                        v_b[:, pj, VCOL : 2 * VCOL],
                        e_prev[:, NB : 2 * NB],
                        start=False,
                        stop=True,
                        skip_group_check=True,
                    )
                    # normalization: denominators at psum row 64
                    stage = sb.tile(
                        [128, NB], F32, tag="stage", bufs=1, name="stage"
                    )
                    nc.vector.tensor_copy(stage[64:65, :], ctx_a[64:65, :])
                    nc.vector.tensor_copy(stage[96:97, :], ctx_b[64:65, :])
                    ctxu = sb.tile([128, NB], F32, tag="ctxu", bufs=2, name="ctxu")
                    nc.vector.tensor_copy(ctxu[0:64, :], ctx_a[0:64, :])
                    nc.vector.tensor_copy(ctxu[64:128, :], ctx_b[0:64, :])
                    sums_p = sb.tile([2, NB], F32, tag="sums", bufs=1, name="sums_p")
                    nc.sync.dma_start(sums_p[0:1, :], stage[64:65, :])
                    nc.sync.dma_start(sums_p[1:2, :], stage[96:97, :])
                    rsum_p = sb.tile([2, NB], F32, tag="rsum", bufs=1, name="rsum_p")
                    nc.vector.reciprocal(out=rsum_p, in_=sums_p)
                    rb = sb.tile([128, NB], F32, tag="rb", bufs=1, name="rb")
                    for sub in range(2):
                        nc.sync.dma_start(
                            rb[sub * 64 : sub * 64 + 64, :],
                            rsum_p[sub : sub + 1, None, :].to_broadcast((1, 64, NB)),
                        )
                    ctxn = sb.tile([128, NB], F16, tag="ctxn", bufs=2, name="ctxn")
                    nc.vector.tensor_mul(out=ctxn, in0=ctxu, in1=rb)

                    # partial out rows -> pout (no bias here; added post-RS)
                    for sti in range(NB // 128):
                        ops = ps.tile([128, QDIM], F32, tag="st", bufs=2, name="ops")
                        for hs in range(2):
                            nc.tensor.matmul(
                                ops[:, hs * 512 : (hs + 1) * 512],
                                ctxn[:, sti * 128 : (sti + 1) * 128],
                                wo_sb[:, hs * 512 : (hs + 1) * 512],
                                start=True,
                                stop=True,
                                skip_group_check=True,
                            )
                        posb = sb.tile(
                            [128, QDIM], F32, tag="posb", bufs=2, name="posb"
                        )
                        nc.vector.tensor_copy(posb, ops)
                        r0 = b * SQ + qb * NB + sti * 128
                        nc.sync.dma_start(pout[r0 : r0 + 128, :], posb)

            # ---- world ReduceScatter of partial outs -> my 1024 rows ----
            nc.gpsimd.collective_compute(
                "ReduceScatter",
                mybir.AluOpType.add,
                replica_groups=[list(range(8))],
                ins=[pout.opt()],
                outs=[rsout.opt()],
            )

            # ---- bias add + int8 row-scale quantization ----
            for i in range(SQS // 128):
                rsf = sb.tile([128, QDIM], F32, tag="posb", bufs=2, name="rsf")
                nc.sync.dma_start(rsf, rsout[i * 128 : (i + 1) * 128, :])
                osf = sb.tile([128, QDIM], F32, tag="osf", bufs=2, name="osf")
                nc.vector.tensor_add(osf, rsf, be_bcast)
                am = sb.tile([128, 1], F32, tag="am", bufs=2, name="am")
                nc.vector.tensor_reduce(
                    out=am,
                    in_=osf,
                    axis=mybir.AxisListType.X,
                    op=mybir.AluOpType.max,
                    apply_absolute_value=True,
                )
                nc.vector.tensor_scalar_max(am, am, 1e-30)
                rr = sb.tile([128, 1], F32, tag="rr", bufs=2, name="rr")
                nc.vector.reciprocal(out=rr, in_=am)
                osb = sb.tile([128, QDIM], I8, tag="osb", bufs=2, name="osb")
                nc.vector.tensor_scalar(
                    out=osb,
                    in0=osf,
                    scalar1=rr[:, 0:1],
                    scalar2=126.0,
                    op0=mybir.AluOpType.mult,
                    op1=mybir.AluOpType.mult,
                )
                nc.sync.dma_start(out_d.ap()[i * 128 : (i + 1) * 128, :], osb)
                nc.sync.dma_start(osc_d.ap()[i * 128 : (i + 1) * 128, :], am)

    nc.compile()
    return nc


_NC_CACHE = {}


def _get_nc():
    if "nc" not in _NC_CACHE:
        _NC_CACHE["nc"] = build_program()
    return _NC_CACHE["nc"]


_IDN = np.eye(128, dtype=np.float16)


def _rowquant_i8(x):
    am = np.maximum(np.abs(x).max(1), 1e-30).astype(np.float32)
    xi = np.rint(x * (126.0 / am)[:, None]).astype(np.int8)
    return xi, (am * (1.0 / 126.0))


def make_in_maps(query, key, value, Wq, bq, Wk, bk, Wv, bv, Wo, bo):
    f16 = np.float16
    q_i8, qsc = _rowquant_i8(query.reshape(SQG, QDIM))
    k_i8, ksc = _rowquant_i8(key.reshape(KVG, KVDIM))
    qk_i8 = np.concatenate([q_i8, k_i8], axis=1)
    qksc = np.stack([qsc, ksc], axis=1).astype(np.float32)
    v16 = value.astype(f16).reshape(KVG, KVDIM)
    bias_eff = (
        bo.astype(np.float64) + bv.astype(np.float64) @ Wo.astype(np.float64)
    ).astype(np.float32)
    beff = bias_eff.reshape(1, QDIM)
    in_maps = []
    for c in range(8):
        cs = slice(c * 128, (c + 1) * 128)
        wpk = np.empty((WPR, 128), f16)
        wpk[0:QDIM] = Wq[:, cs]
        wpk[QDIM : QDIM + KVDIM] = Wk[:, cs]
        wpk[QDIM + KVDIM : QDIM + 2 * KVDIM] = Wv[:, cs]
        wpk[QDIM + 2 * KVDIM :] = (
            Wo[cs, :].astype(f16).reshape(KQ * 128, 128)
        )
        aux = np.stack([bq[cs], bk[cs]]).astype(np.float32)
        in_maps.append(
            dict(
                qksh=qk_i8[c * SQS : (c + 1) * SQS],
                qksc=qksc[c * SQS : (c + 1) * SQS],
                vsh=v16[c * SQS : (c + 1) * SQS],
                wp=wpk,
                aux=aux,
                beff=beff,
                idn=_IDN,
            )
        )
    return in_maps


def kernel(query, key, value, Wq, bq, Wk, bk, Wv, bv, Wo, bo, _trace=False):
    nc = _get_nc()
    in_maps = make_in_maps(query, key, value, Wq, bq, Wk, bk, Wv, bv, Wo, bo)
    try:
        res = run_bass_kernel_spmd(
            nc, in_maps, core_ids=list(range(8)), trace=_trace
        )
    except Exception:
        # one retry for transient axon-tunnel drops
        res = run_bass_kernel_spmd(
            nc, in_maps, core_ids=list(range(8)), trace=_trace
        )
    out = np.empty((SQG, QDIM), np.float32)
    for c in range(8):
        sc = res.results[c]["osc"].astype(np.float32) * (1.0 / 126.0)
        np.multiply(
            res.results[c]["out"],
            sc,
            out=out[c * SQS : (c + 1) * SQS],
            dtype=np.float32,
        )
    out = out.reshape(B, SQ, QDIM)
    if _trace:
        return out, res
    return out
